# revision 13
# baseline (speedup 1.0000x reference)
"""Trainium2 Bass kernel v7 for nn_ContentLoss (Gaussian-blur content MSE).

Math: MSE( blur61(a).mean(ch), blur61(b).mean(ch) ) with a 61x61 Gaussian
(sigma=1, separable, effective support well inside +-16 taps) and reflect
padding.  Everything before the final square is linear, so each core
computes g = colconv(rowconv(mean_ch(a - b))) per image as two banded
matmuls on the TensorEngine.

The kernel is DMA-bound (12.6 MB of fp32 input per core, ~425 GB/s fabric
rate).  v7 lessons from the v5/v6 traces:

  * ALL transfers ride the HWDGE (sync) queue as fp32 -- SWDGE descriptor
    emission (~1.6 us/op on the Q7) paced the v5/v6 streams and its ring
    throttle added multi-us issue gaps.  HWDGE issue is ~0.7 us on the
    otherwise-idle Sync engine and trivially stays ahead of the drain.
  * h-chunk-contiguous layout (h = 128c + p) makes pass-1 BANDED: chunk c
    only writes h_out columns [128c-16, 128c+144).  Pass-1 output lives in
    PER-COLUMN-GROUP PSUM banks, so y1 copies, pass-2 and squares pipeline
    per group without PE-write/ACT-read bank collisions.
  * image 1 streams C-QUARTER-MAJOR (all six planes' c-slice together), so
    each quarter's combine -> pass-1 -> y1 -> pass-2 -> squares completes
    DURING the stream; post-last-byte work is only the c3 slice.
  * the channel combine runs on the DVE in fp32 (1x); the b2 subs write
    bf16, so the cast the matmuls need is free.  All matmuls are bf16.
  * one-matmul PE keep-warm fillers after plane DMAs hold the HAM clock
    gate at 2.4 GHz.

PSUM budget (8 banks): 3 (pass-1 groups 0-2) + 2 (group 3 split by wc
pair) + 2 (pass-2 rotating) + 1 (filler) = 8.

Sharding: pure data parallel, 2 images per core across 8 cores.  Each core
returns per-partition partial sums of g**2; the host reduces in float64.
"""

import sys

import numpy as np

if "/opt/trn_rl_repo" not in sys.path:
    sys.path.insert(0, "/opt/trn_rl_repo")

N = 512
P = 128
IMGS_PER_CORE = 2
N_CORES = 8
FULL_BATCH = 16

S = 16  # pass-1 window halo (true band is +-30 but taps beyond +-8 are ~0)
W1 = [(max(0, 128 * c - S), min(N, 128 * c + 128 + S)) for c in range(4)]
R1OFF = [0, 144, 304, 464]
R1TOT = 608
# column groups: G[q] = [GB[q], GB[q+1]) closes after chunk q's segments
GB = [0, 112, 240, 368, 512]
GW = [112, 128, 128, 144]

_F = lambda wc: False
# pass-1 segment table: per c, (global_s0, global_s1, qbank, local_lo,
# start(wc), stop(wc)).  One start per bank (clears the whole bank), one
# stop on the bank's last matmul.
SEGT = [
    [
        (0, 112, 0, 0, lambda wc: wc == 0, lambda wc: wc == 3),
        (112, 144, 1, 0, lambda wc: wc == 0, _F),
    ],
    [
        (112, 144, 1, 0, _F, _F),
        (144, 240, 1, 32, _F, lambda wc: wc == 3),
        (240, 272, 2, 0, lambda wc: wc == 0, _F),
    ],
    [
        (240, 272, 2, 0, _F, _F),
        (272, 368, 2, 32, _F, lambda wc: wc == 3),
        (368, 400, 3, 0, lambda wc: wc in (0, 2), _F),
    ],
    [
        (368, 400, 3, 0, _F, _F),
        (400, 512, 3, 32, _F, lambda wc: wc in (1, 3)),
    ],
]


def _build_B():
    """B[i, j]: 1D 61-tap normalized Gaussian conv matrix with reflect pad."""
    x = np.arange(61, dtype=np.float64)
    k1 = np.exp(-((x - 30.0) ** 2) / 2.0)
    k1n = k1 / k1.sum()
    B = np.zeros((N, N), np.float64)
    i = np.arange(N)
    for t in range(61):
        j = i + (t - 30)
        j = np.abs(j)
        j = np.where(j > N - 1, 2 * (N - 1) - j, j)
        np.add.at(B, (i, j), k1n[t])
    return B


def _build_consts_v6():
    """Packed [128, 1376] bf16: r1w (608) | c2main (512) | c2prev | c2next.

    r1w_c[k, j]          = B[W1[c].lo + j, 128c + k] / 3   (windowed pass-1)
    c2main[k, 128m + j]  = BT[128m + k, 128m + j]
    c2prev[k, j]         = BT[k, 128 + j]   (interior Toeplitz, fits all m)
    c2next[k, j]         = BT[256 + k, 128 + j]
    """
    import ml_dtypes

    B = _build_B()
    BT = B.T
    pack = np.zeros((P, 1376), np.float64)
    for c, (lo, hi) in enumerate(W1):
        pack[:, R1OFF[c] : R1OFF[c] + (hi - lo)] = (
            BT[128 * c : 128 * (c + 1), lo:hi] / 3.0
        )
    for m in range(4):
        pack[:, R1TOT + 128 * m : R1TOT + 128 * (m + 1)] = BT[
            128 * m : 128 * (m + 1), 128 * m : 128 * (m + 1)
        ]
    pack[:, 1120:1248] = BT[0:128, 128:256]
    pack[:, 1248:1376] = BT[256:384, 128:256]
    return pack.astype(ml_dtypes.bfloat16)


def build_nc():
    from contextlib import ExitStack

    import concourse.bacc as bacc
    import concourse.tile as tile
    from concourse import mybir

    f32 = mybir.dt.float32
    bf16 = mybir.dt.bfloat16
    nc = bacc.Bacc(
        "TRN2", target_bir_lowering=False, debug=False, num_devices=N_CORES
    )

    a = nc.dram_tensor("a", [IMGS_PER_CORE, 3, N, N], f32, kind="ExternalInput")
    b = nc.dram_tensor("b", [IMGS_PER_CORE, 3, N, N], f32, kind="ExternalInput")
    # out[p, img*8 + q*2 + pair] = partial sums of g**2 over that column
    # group / m-pair; col 16 is PE-filler garbage (excluded on host).
    out = nc.dram_tensor("out", [P, 17], f32, kind="ExternalOutput")

    consts_d = nc.inline_tensor(_build_consts_v6(), name="constpack")

    with tile.TileContext(nc) as tc, ExitStack() as ctx:
        consts = ctx.enter_context(tc.tile_pool(name="consts", bufs=1))
        planes = ctx.enter_context(tc.tile_pool(name="planes", bufs=1))
        dpool = ctx.enter_context(tc.tile_pool(name="dpool", bufs=2))
        y1pool = ctx.enter_context(tc.tile_pool(name="y1pool", bufs=2))
        accp = ctx.enter_context(tc.tile_pool(name="accp", bufs=1))
        scratchp = ctx.enter_context(tc.tile_pool(name="scratchp", bufs=1))
        psum1 = ctx.enter_context(tc.tile_pool(name="psum1", bufs=1, space="PSUM"))
        psum2 = ctx.enter_context(tc.tile_pool(name="psum2", bufs=2, space="PSUM"))
        psumw = ctx.enter_context(tc.tile_pool(name="psumw", bufs=1, space="PSUM"))

        ct = consts.tile([P, 1376], bf16, name="ct")

        def r1w(c, s0, s1):
            lo = W1[c][0]
            return ct[:, R1OFF[c] + s0 - lo : R1OFF[c] + s1 - lo]

        def c2main(m):
            return ct[:, R1TOT + 128 * m : R1TOT + 128 * (m + 1)]

        c2prev = ct[:, 1120:1248]
        c2next = ct[:, 1248:1376]

        acc_t = accp.tile([P, 17], f32, name="acc_t")

        psw = psumw.tile([P, N], f32, name="psw")

        def filler(lhsT, rhs):
            """Single-matmul keep-warm group into the shared psw bank."""
            nc.tensor.matmul(
                psw[:, 0 : rhs.free_size()],
                lhsT=lhsT,
                rhs=rhs,
                start=True,
                stop=True,
            )

        def pass1_segs(d, img, q012, q3ab, c):
            for wc in range(4):
                for s0, s1, qb, llo, fst, fsp in SEGT[c]:
                    po = (
                        q012[qb][:, wc, llo : llo + s1 - s0]
                        if qb < 3
                        else q3ab[wc // 2][:, wc % 2, llo : llo + s1 - s0]
                    )
                    nc.tensor.matmul(
                        po,
                        lhsT=d[:, c, 128 * wc : 128 * (wc + 1)],
                        rhs=r1w(c, s0, s1),
                        start=fst(wc),
                        stop=fsp(wc),
                    )

        def group_pipeline(img, q, q012, q3ab, y1):
            """y1 copies + pass-2 + squares for column group q (= chunk q)."""
            g0, g1, gw = GB[q], GB[q + 1], GW[q]
            for wc in range(4):
                src = (
                    q012[q][:, wc, 0:gw]
                    if q < 3
                    else q3ab[wc // 2][:, wc % 2, 0:gw]
                )
                if (wc + q) % 2 == 0:
                    nc.scalar.copy(y1[:, wc, g0:g1], src)
                else:
                    nc.vector.tensor_copy(y1[:, wc, g0:g1], src)

            for pair in range(2):
                ps2 = psum2.tile(
                    [P, 2, 256], f32, name=f"ps2_{img}_{q}_{pair}", tag="ps2"
                )
                ms = (0, 1) if pair == 0 else (2, 3)
                seq = []
                for m in ms:
                    if m > 0:
                        seq.append((c2prev, m - 1, m))
                for m in ms:
                    if m < 3:
                        seq.append((c2next, m + 1, m))
                for m in ms:
                    seq.append((c2main(m), m, m))
                for i, (lhs_c, msrc, mdst) in enumerate(seq):
                    nc.tensor.matmul(
                        ps2[:, mdst % 2, 0:gw],
                        lhsT=lhs_c,
                        rhs=y1[:, msrc, g0:g1],
                        start=(i == 0),
                        stop=(i == len(seq) - 1),
                    )
                col = 8 * img + 2 * q + pair
                scr = scratchp.tile(
                    [P, 2, 144],
                    f32,
                    name=f"scr_{img}_{q}_{pair}",
                    tag="scr",
                    bufs=3,
                )
                nc.scalar.activation(
                    scr[:, :, 0:gw],
                    ps2[:, :, 0:gw],
                    mybir.ActivationFunctionType.Square,
                    accum_out=acc_t[:, col : col + 1],
                )

        # ================= image 0: plane-major stream =================
        img = 0
        plane_ts = []
        for pi, (src, src_name, ch) in enumerate(
            (s, n_, c) for s, n_ in ((a, "a"), (b, "b")) for c in range(3)
        ):
            pt = planes.tile(
                [P, 4, N], f32, name=f"pl_{src_name}0c{ch}", tag="pl", bufs=6
            )
            src_ap = src.ap()[img, ch].rearrange("(c p) w -> p c w", p=P)
            nc.sync.dma_start(out=pt, in_=src_ap)
            filler(pt[:, 0, 0:128], pt[:, 1, 0:128])
            plane_ts.append(pt)
            if pi == 1:
                nc.sync.dma_start(out=ct, in_=consts_d.ap())

        d0 = dpool.tile([P, 4, N], bf16, name="d_0", tag="d")
        dt0 = dpool.tile([P, 4, N], f32, name="dt_0", tag="dt")
        nc.vector.tensor_add(dt0, plane_ts[0], plane_ts[1])
        nc.vector.tensor_add(dt0, dt0, plane_ts[2])
        nc.vector.tensor_sub(dt0, dt0, plane_ts[3])
        nc.vector.tensor_sub(dt0, dt0, plane_ts[4])
        # final sub casts to bf16 for the matmuls, per c-slice
        for c in range(4):
            nc.vector.tensor_sub(
                d0[:, c, :], dt0[:, c, :], plane_ts[5][:, c, :]
            )

        q012_0 = [
            psum1.tile([P, 4, 128], f32, name=f"p1_0_q{q}", tag="q012", bufs=3)
            for q in range(3)
        ]
        q3ab_0 = [
            psum1.tile([P, 2, 256], f32, name=f"p1_0_q3{s_}", tag="q3", bufs=2)
            for s_ in ("a", "b")
        ]
        y1_0 = y1pool.tile([P, 4, N], bf16, name="y1_0", tag="y1")
        for c in range(4):
            pass1_segs(d0, 0, q012_0, q3ab_0, c)
            group_pipeline(0, c, q012_0, q3ab_0, y1_0)

        # ================= image 1: c-quarter-major stream =============
        img = 1
        srcs = [(s, n_, c) for s, n_ in ((a, "a"), (b, "b")) for c in range(3)]
        pl1 = [
            planes.tile(
                [P, 4, N], f32, name=f"pl_{nm}1c{ch}", tag="pl1", bufs=6
            )
            for (_s, nm, ch) in srcs
        ]
        d1 = dpool.tile([P, 4, N], bf16, name="d_1", tag="d")
        dt1 = dpool.tile([P, 4, N], f32, name="dt_1", tag="dt")
        q012_1 = [
            psum1.tile([P, 4, 128], f32, name=f"p1_1_q{q}", tag="q012", bufs=3)
            for q in range(3)
        ]
        q3ab_1 = [
            psum1.tile([P, 2, 256], f32, name=f"p1_1_q3{s_}", tag="q3", bufs=2)
            for s_ in ("a", "b")
        ]
        y1_1 = y1pool.tile([P, 4, N], bf16, name="y1_1", tag="y1")

        for c in range(4):
            # six plane c-slices (b2's c3 slice in w-halves)
            for pi, (src, _nm, ch) in enumerate(srcs):
                src_ap = src.ap()[img, ch].rearrange("(c p) w -> p c w", p=P)
                if pi >= 1 and c == 3:
                    for wh in range(2):
                        ws = slice(256 * wh, 256 * (wh + 1))
                        nc.sync.dma_start(
                            out=pl1[pi][:, c, ws], in_=src_ap[:, c, ws]
                        )
                else:
                    nc.sync.dma_start(
                        out=pl1[pi][:, c, :], in_=src_ap[:, c, :]
                    )
                if c < 3 and pi < 5:
                    filler(pl1[pi][:, c, 0:128], pl1[pi][:, c, 128:256])
            # combine this quarter (fp32 1x on DVE; last sub casts bf16).
            # c3 runs per w-half so every link paces with its half-DMA and
            # only the final half-sub trails the last byte.
            cs = slice(c, c + 1)
            if c == 3:
                for op in range(5):
                    for wh in range(2):
                        ws = slice(256 * wh, 256 * (wh + 1))
                        if op == 0:
                            nc.vector.tensor_add(
                                dt1[:, cs, ws],
                                pl1[0][:, cs, ws],
                                pl1[1][:, cs, ws],
                            )
                        elif op in (1,):
                            nc.vector.tensor_add(
                                dt1[:, cs, ws],
                                dt1[:, cs, ws],
                                pl1[2][:, cs, ws],
                            )
                        elif op in (2, 3):
                            nc.vector.tensor_sub(
                                dt1[:, cs, ws],
                                dt1[:, cs, ws],
                                pl1[op + 1][:, cs, ws],
                            )
                        else:
                            nc.vector.tensor_sub(
                                d1[:, c, ws], dt1[:, c, ws], pl1[5][:, c, ws]
                            )
            else:
                nc.vector.tensor_add(
                    dt1[:, cs, :], pl1[0][:, cs, :], pl1[1][:, cs, :]
                )
                nc.vector.tensor_add(
                    dt1[:, cs, :], dt1[:, cs, :], pl1[2][:, cs, :]
                )
                nc.vector.tensor_sub(
                    dt1[:, cs, :], dt1[:, cs, :], pl1[3][:, cs, :]
                )
                nc.vector.tensor_sub(
                    dt1[:, cs, :], dt1[:, cs, :], pl1[4][:, cs, :]
                )
                nc.vector.tensor_sub(
                    d1[:, c, :], dt1[:, c, :], pl1[5][:, c, :]
                )
            # this quarter's pass-1 + the column group it closes
            pass1_segs(d1, 1, q012_1, q3ab_1, c)
            group_pipeline(1, c, q012_1, q3ab_1, y1_1)

        # consume the filler bank so the keep-warm matmuls stay live
        scrw = scratchp.tile([P, N], f32, name="scrw", tag="scrw", bufs=1)
        nc.scalar.activation(
            scrw[:, 0:128],
            psw[:, 0:128],
            mybir.ActivationFunctionType.Square,
            accum_out=acc_t[:, 16:17],
        )

        nc.sync.dma_start(out=out.ap(), in_=acc_t)

    nc.finalize()
    return nc


_CACHE = {}


def _get_nc():
    if "nc" not in _CACHE:
        _CACHE["nc"] = build_nc()
    return _CACHE["nc"]


def run(inputs, **spmd_kwargs):
    """Run on 8 cores; returns (scalar_result, BassKernelResults)."""
    from concourse import bass_utils

    a = np.ascontiguousarray(np.asarray(inputs["a"], dtype=np.float32))
    b = np.ascontiguousarray(np.asarray(inputs["b"], dtype=np.float32))
    assert a.shape == (FULL_BATCH, 3, N, N) and b.shape == a.shape

    nc = _get_nc()
    in_maps = []
    for core in range(N_CORES):
        sl = slice(core * IMGS_PER_CORE, (core + 1) * IMGS_PER_CORE)
        in_maps.append(
            {
                "a": np.ascontiguousarray(a[sl]),
                "b": np.ascontiguousarray(b[sl]),
            }
        )
    res = bass_utils.run_bass_kernel_spmd(
        nc, in_maps, core_ids=list(range(N_CORES)), **spmd_kwargs
    )
    total = 0.0
    for r in res.results:
        total += np.asarray(r["out"])[:, :16].astype(np.float64).sum()
    mse = np.float32(total / (FULL_BATCH * N * N))
    return np.asarray(mse, dtype=np.float32), res


# ---------------------------------------------------------------------------
# Fallback: kernel v4 (fp32 DVE combine + f32r matmuls) — known-good on HW.
# Used only if the primary path fails to compile/run for any reason.


def _v4_build_consts():
    B = _build_B()
    R1 = np.zeros((P, 4, N), np.float16)
    for c in range(4):
        R1[:, c, :] = (B[:, c::4].T / 3.0).astype(np.float16)
    BT = B.T
    c2main = np.zeros((P, 4, 128), np.float16)
    for m in range(4):
        c2main[:, m, :] = BT[128 * m : 128 * (m + 1), 128 * m : 128 * (m + 1)]
    c2prev = BT[0:128, 128:256].astype(np.float16)
    c2next = BT[256:384, 128:256].astype(np.float16)
    return R1, c2main, c2prev, c2next


def _v4_build_nc():
    from contextlib import ExitStack

    import concourse.bacc as bacc
    import concourse.tile as tile
    from concourse import mybir

    f32 = mybir.dt.float32
    f16 = mybir.dt.float16
    f32r = mybir.dt.float32r
    nc = bacc.Bacc(
        "TRN2", target_bir_lowering=False, debug=False, num_devices=N_CORES
    )

    a = nc.dram_tensor("a", [IMGS_PER_CORE, 3, N, N], f32, kind="ExternalInput")
    b = nc.dram_tensor("b", [IMGS_PER_CORE, 3, N, N], f32, kind="ExternalInput")
    out = nc.dram_tensor(
        "out", [P, 4 * IMGS_PER_CORE + 1], f32, kind="ExternalOutput"
    )

    R1_np, c2main_np, c2prev_np, c2next_np = _v4_build_consts()
    R1_d = nc.inline_tensor(R1_np, name="R1")
    c2main_d = nc.inline_tensor(c2main_np, name="c2main")
    c2prev_d = nc.inline_tensor(c2prev_np, name="c2prev")
    c2next_d = nc.inline_tensor(c2next_np, name="c2next")

    with tile.TileContext(nc) as tc, ExitStack() as ctx:
        consts = ctx.enter_context(tc.tile_pool(name="consts", bufs=1))
        planes = ctx.enter_context(tc.tile_pool(name="planes", bufs=12))
        dpool = ctx.enter_context(tc.tile_pool(name="dpool", bufs=2))
        y1pool = ctx.enter_context(tc.tile_pool(name="y1pool", bufs=2))
        accp = ctx.enter_context(tc.tile_pool(name="accp", bufs=1))
        scratchp = ctx.enter_context(tc.tile_pool(name="scratchp", bufs=2))
        psum1 = ctx.enter_context(tc.tile_pool(name="psum1", bufs=4, space="PSUM"))
        psum2 = ctx.enter_context(tc.tile_pool(name="psum2", bufs=3, space="PSUM"))
        psumw = ctx.enter_context(tc.tile_pool(name="psumw", bufs=1, space="PSUM"))

        r1_h = consts.tile([P, 4, N], f16, name="r1_h")
        nc.sync.dma_start(out=r1_h, in_=R1_d.ap())
        c2main_h = consts.tile([P, 4, 128], f16, name="c2main_h")
        nc.sync.dma_start(out=c2main_h, in_=c2main_d.ap())
        c2prev_h = consts.tile([P, 128], f16, name="c2prev_h")
        nc.sync.dma_start(out=c2prev_h, in_=c2prev_d.ap())
        c2next_h = consts.tile([P, 128], f16, name="c2next_h")
        nc.sync.dma_start(out=c2next_h, in_=c2next_d.ap())

        r1_t = consts.tile([P, 4, N], f32r, name="r1_t")
        nc.scalar.copy(r1_t, r1_h)
        c2main_t = consts.tile([P, 4, 128], f32r, name="c2main_t")
        nc.scalar.copy(c2main_t, c2main_h)
        c2prev_t = consts.tile([P, 128], f32r, name="c2prev_t")
        nc.scalar.copy(c2prev_t, c2prev_h)
        c2next_t = consts.tile([P, 128], f32r, name="c2next_t")
        nc.scalar.copy(c2next_t, c2next_h)

        acc_t = accp.tile([P, 4 * IMGS_PER_CORE + 1], f32, name="acc_t")

        psw = psumw.tile([P, N], f32, name="psw")

        def filler(lhsT, rhs):
            nc.tensor.matmul(
                psw[:, 0 : rhs.free_size()],
                lhsT=lhsT,
                rhs=rhs,
                start=True,
                stop=True,
            )

        filler(r1_h[:, 0, 0:128], r1_h[:, 1, 0:128])
        filler(r1_h[:, 2, 0:128], r1_h[:, 3, 0:128])

        for img in range(IMGS_PER_CORE):
            plane_ts = []
            for pi, (src, src_name, ch) in enumerate(
                (s, n_, c) for s, n_ in ((a, "a"), (b, "b")) for c in range(3)
            ):
                pt = planes.tile(
                    [P, 4, N], f32, name=f"pl_{src_name}{img}c{ch}", tag="pl"
                )
                src_ap = src.ap()[img, ch].rearrange("(p c) w -> p c w", p=P)
                if pi == 4:
                    nc.sync.dma_start(out=pt[:, 0:2, :], in_=src_ap[:, 0:2, :])
                    nc.sync.dma_start(out=pt[:, 2:4, :], in_=src_ap[:, 2:4, :])
                elif pi == 5:
                    for c in range(4):
                        nc.sync.dma_start(
                            out=pt[:, c, :], in_=src_ap[:, c, :]
                        )
                else:
                    nc.sync.dma_start(out=pt, in_=src_ap)
                if pi < 4:
                    filler(pt[:, 0, 0:128], pt[:, 1, 0:128])
                plane_ts.append(pt)

            d = dpool.tile([P, 4, N], f32r, name=f"d_{img}", tag="d")
            nc.vector.tensor_add(d, plane_ts[0], plane_ts[1])
            nc.vector.tensor_add(d, d, plane_ts[2])
            nc.vector.tensor_sub(d, d, plane_ts[3])
            for half in range(2):
                hs = slice(2 * half, 2 * half + 2)
                nc.vector.tensor_sub(
                    d[:, hs, :], d[:, hs, :], plane_ts[4][:, hs, :]
                )
            for c in range(4):
                nc.vector.tensor_sub(
                    d[:, c, :], d[:, c, :], plane_ts[5][:, c, :]
                )

            ps1 = [
                psum1.tile([P, N], f32, name=f"ps1_{img}_{wc}", tag="ps1")
                for wc in range(4)
            ]
            for c in range(4):
                for wc in range(4):
                    nc.tensor.matmul(
                        ps1[wc],
                        lhsT=d[:, c, 128 * wc : 128 * (wc + 1)],
                        rhs=r1_t[:, c, :],
                        start=(c == 0),
                        stop=(c == 3),
                    )
            y1 = y1pool.tile([P, 4, N], f32r, name=f"y1_{img}", tag="y1")
            for wc in range(4):
                if wc % 2 == 0:
                    nc.scalar.copy(y1[:, wc, :], ps1[wc])
                else:
                    nc.vector.tensor_copy(y1[:, wc, :], ps1[wc])

            for m in range(4):
                ps2 = psum2.tile([P, N], f32, name=f"ps2_{img}_{m}", tag="ps2")
                first = True
                if m > 0:
                    nc.tensor.matmul(
                        ps2,
                        lhsT=c2prev_t,
                        rhs=y1[:, m - 1, :],
                        start=first,
                        stop=False,
                    )
                    first = False
                if m < 3:
                    nc.tensor.matmul(
                        ps2,
                        lhsT=c2next_t,
                        rhs=y1[:, m + 1, :],
                        start=first,
                        stop=False,
                    )
                    first = False
                nc.tensor.matmul(
                    ps2,
                    lhsT=c2main_t[:, m, :],
                    rhs=y1[:, m, :],
                    start=False,
                    stop=True,
                )
                scr = scratchp.tile([P, N], f32, name=f"scr_{img}_{m}", tag="scr")
                nc.scalar.activation(
                    scr,
                    ps2,
                    mybir.ActivationFunctionType.Square,
                    accum_out=acc_t[:, 4 * img + m : 4 * img + m + 1],
                )

        scrw = scratchp.tile([P, N], f32, name="scrw", tag="scr")
        nc.scalar.activation(
            scrw[:, 0:128],
            psw[:, 0:128],
            mybir.ActivationFunctionType.Square,
            accum_out=acc_t[:, 8:9],
        )

        nc.sync.dma_start(out=out.ap(), in_=acc_t)

    nc.finalize()
    return nc


def _v4_run(inputs, **spmd_kwargs):
    from concourse import bass_utils

    a = np.ascontiguousarray(np.asarray(inputs["a"], dtype=np.float32))
    b = np.ascontiguousarray(np.asarray(inputs["b"], dtype=np.float32))
    assert a.shape == (FULL_BATCH, 3, N, N) and b.shape == a.shape

    if "v4" not in _CACHE:
        _CACHE["v4"] = _v4_build_nc()
    nc = _CACHE["v4"]
    in_maps = []
    for core in range(N_CORES):
        sl = slice(core * IMGS_PER_CORE, (core + 1) * IMGS_PER_CORE)
        in_maps.append(
            {
                "a": np.ascontiguousarray(a[sl]),
                "b": np.ascontiguousarray(b[sl]),
            }
        )
    res = bass_utils.run_bass_kernel_spmd(
        nc, in_maps, core_ids=list(range(N_CORES)), **spmd_kwargs
    )
    total = 0.0
    for r in res.results:
        total += np.asarray(r["out"])[:, :8].astype(np.float64).sum()
    mse = np.float32(total / (FULL_BATCH * N * N))
    return np.asarray(mse, dtype=np.float32), res


def kernel(**inputs) -> np.ndarray:
    try:
        result, _ = run(inputs)
        return result
    except Exception:
        import traceback

        traceback.print_exc()
        result, _ = _v4_run(inputs)
        return result
